# revision 11
# baseline (speedup 1.0000x reference)
"""Distributed Trainium2 kernel for AdaptiveGraphRecursiveConvolution.

Math (reference):
    out = relu( sum_g mix_w[g] * sum_k A_{gk} @ (h @ W[g,k])
              + sum_g inp_mix_w[g] * sum_k A_{gk} @ (x @ inp_W[g,k]) )

Folding the scalar mixing weights into the dense weights and merging the
h/x paths gives, with S = G*K edge sets and V_s = [mix_w*W_s ; inp_mix*inp_W_s]:
    out = relu( sum_s A_s @ (hx @ V_s) ),  hx = [h | x]  (N x 2F)

Device strategy (8 NeuronCores, SPMD single graph, per-core data):
  - dst-nodes sharded: core c owns rows [c*NSH, (c+1)*NSH).
  - Phase A (replicated): pre2 = hx @ [V_0..V_3]  ([NP2, S*128] bf16) via PE,
    streamed to private DRAM. Node supertiles are emitted with the two
    int16-index halves INTERLEAVED so both halves become gatherable early.
  - Phase B: per edge-set SpMM. Edges (pre-sharded/sorted/padded on host) are
    gathered per-edge from pre2 via SWDGE dma_gather (bf16 rows), and
    scatter-added into PSUM dst regions via one-hot matmuls on PE:
        psum[region(dst_tile)] += T_chunk^T @ msg_chunk
    where T[e, j] = val_e * (dst_loc_e == j) is built on DVE with one fused
    tensor_scalar (is_equal then mult) per 128-edge subchunk.
  - PSUM packing: 4 dst tiles per bank (regions), matmuls never use
    start=True on phase-B banks; regions are value-zeroed before reuse so
    accumulate-or-overwrite is correct either way. Tiles are grouped in
    super-groups of 24 (6 banks x 4 regions); calls within a super-group are
    sorted by pre2-readiness level so phase B streams during phase A.
  - relu on ACT from PSUM, DMA out. Host concatenates the 8 shards.
"""

import os
import sys
import time

import numpy as np

sys.path.insert(0, "/opt/trn_rl_repo")
sys.path.insert(0, "/root/.axon_site/_ro/trn_rl_repo")

import ml_dtypes  # noqa: E402

BF16 = ml_dtypes.bfloat16


# ---------------------------------------------------------------- config ---

class Cfg:
    def __init__(self, N, E, S=4, M=8, HALF=None, SUP=None, GCH=16, SGT=24,
                 RB=8, RBTB=8, NQ=4, NLV=8):
        self.N, self.E, self.S, self.M = N, E, S, M
        self.F2 = 256          # hx feature dim (2*128)
        self.O = 128
        self.NSH = N // M      # dst rows owned per core
        self.NPAD = _ru(self.NSH, 128)
        self.NT = self.NPAD // 128          # dst tiles per core
        self.SGT = SGT                      # tiles per super-group (<= 24)
        self.SGS = [list(range(a, min(a + SGT, self.NT)))
                    for a in range(0, self.NT, SGT)]
        self.HALF = HALF if HALF else _ru(-(-N // 2), 128)
        assert self.HALF <= 32767
        self.NP2 = 2 * self.HALF            # padded node rows in pre2
        assert N <= self.NP2
        self.SUP = SUP if SUP else 2560     # nodes per phase-A supertile
        assert self.SUP % 128 == 0 and self.NP2 % self.SUP == 0
        self.NSUP = self.NP2 // self.SUP
        assert self.NSUP % 2 == 0
        self.T20 = self.SUP // 128
        self.NTA = self.NP2 // 128          # phase-A node tiles
        self.GCH = GCH                      # max subchunks (128 edges) per gather
        self.RB = RB                        # msg ring depth (calls)
        self.RBTB = RBTB                    # T ring depth (calls)
        self.NQ = NQ                        # SWDGE queues
        self.NLV = NLV                      # pre2 readiness levels
        self.GRP = 8                        # tiles per pre2 write group
        self.NTH = self.HALF // 128         # node tiles per half
        self.NGH = self.NTH // self.GRP     # write groups per half
        self.NG = 2 * self.NGH
        assert self.NG % NLV == 0


def _ru(x, m):
    return (x + m - 1) // m * m


FULL = Cfg(N=40000, E=640000, HALF=20480, SUP=2560)


# ------------------------------------------------------------- host prep ---

def _fold_weights(W, inp_W, mix_w, inp_mix_w, C):
    """Return v0, v1: [128, S*128] bf16 (h-path and x-path stationary weights)."""
    G, K = W.shape[0], W.shape[1]
    S = G * K
    Wm = (W.astype(np.float64) * mix_w.astype(np.float64)[:, None, None, None])
    Im = (inp_W.astype(np.float64) * inp_mix_w.astype(np.float64)[:, None, None, None])
    v0 = Wm.reshape(S, 128, 128).transpose(1, 0, 2).reshape(128, S * 128)
    v1 = Im.reshape(S, 128, 128).transpose(1, 0, 2).reshape(128, S * 128)
    return v0.astype(BF16), v1.astype(BF16)


def _phase_a_meta(C):
    """Emission order of phase-A supertiles/groups with interleaved halves."""
    H = C.NSUP // 2
    sup_order = []
    for k in range(H):
        sup_order += [k, H + k]
    # em[j]: emission index of phase-A node tile j (j = NP2-row-block index)
    em = np.zeros(C.NTA, dtype=np.int64)
    for pos, sup in enumerate(sup_order):
        for t20 in range(C.T20):
            em[sup * C.T20 + t20] = pos * C.T20 + t20
    tile_of_em = np.zeros(C.NTA, dtype=np.int64)
    tile_of_em[em] = np.arange(C.NTA)
    # groups (h, gi): tiles jh = gi*GRP .. +GRP-1 of half h
    groups = [(h, gi) for h in range(2) for gi in range(C.NGH)]
    em_last = {}
    for (h, gi) in groups:
        tiles = [h * C.NTH + gi * C.GRP + q for q in range(C.GRP)]
        em_last[(h, gi)] = int(max(em[t] for t in tiles))
    grp_order = sorted(groups, key=lambda g: em_last[g])
    rank = {g: i for i, g in enumerate(grp_order)}
    per_lvl = C.NG // C.NLV
    lvl = {g: rank[g] // per_lvl for g in groups}
    return dict(sup_order=sup_order, em=em, tile_of_em=tile_of_em,
                grp_order=grp_order, rank=rank, lvl=lvl, em_last=em_last)


def _prep_edges(edge_src, edge_dst, edge_val, C, pa):
    """Shard/sort/pad edges. Returns (sched, per_core)."""
    S, E, M = C.S, C.E, C.M
    src = edge_src.reshape(S, E).astype(np.int64)
    dst = edge_dst.reshape(S, E).astype(np.int64)
    val = edge_val.reshape(S, E).astype(np.float32)

    NTH = C.NTH
    lvl = pa["lvl"]
    # raw[c][s][t][h] = (idx16, dstloc, val); pre2 rows are PERMUTED within
    # each half: node (p=sl%128, j=sl//128) is stored at row p*NTH + j.
    raw = [[[[None, None] for _ in range(C.NT)] for _ in range(S)] for _ in range(M)]
    cnt = np.zeros((M, S, C.NT, 2), dtype=np.int64)
    for s in range(S):
        core_of = dst[s] // C.NSH
        for c in range(M):
            sel = np.nonzero(core_of == c)[0]
            d = dst[s][sel] - c * C.NSH
            t = d // 128
            h = (src[s][sel] >= C.HALF).astype(np.int64)
            # secondary sort by src so low-src subchunks can gather early
            key = (t * 2 + h) * (2 * C.HALF) + src[s][sel]
            order = np.argsort(key, kind="stable")
            sel, d, t, h = sel[order], d[order], t[order], h[order]
            key = key[order]
            bounds = np.searchsorted(key, np.arange(C.NT * 2 + 1) * (2 * C.HALF))
            for ti in range(C.NT):
                for hi in range(2):
                    a, b = bounds[ti * 2 + hi], bounds[ti * 2 + hi + 1]
                    ss = sel[a:b]
                    sl = src[s][ss] - hi * C.HALF
                    raw[c][s][ti][hi] = (
                        ((sl % 128) * NTH + sl // 128).astype(np.int16),
                        (d[a:b] - ti * 128).astype(np.int16),
                        val[s][ss],
                    )
                    cnt[c, s, ti, hi] = b - a

    # common padded lengths
    L = np.maximum(cnt.max(axis=0), 1)
    L = ((L + 127) // 128 * 128)  # [S, NT, 2]

    # Per-(s,t,h) per-subchunk readiness class: max over cores of the pre2
    # write-group LEVEL of the rows the subchunk gathers.
    qcls = {}
    for s in range(S):
        for t in range(C.NT):
            for h in range(2):
                nsub = int(L[s][t][h]) // 128
                cls = []
                for j in range(nsub):
                    m = 0
                    for c in range(M):
                        seg_i = raw[c][s][t][h][0][j * 128:(j + 1) * 128]
                        if len(seg_i):
                            gi_max = int((seg_i % NTH).max()) // C.GRP
                            # rows in this subchunk span groups up to gi_max;
                            # level needed = max level among groups 0..gi_max
                            # (sorted-by-src makes gi_max the binding one, but
                            # levels are not monotone in gi -> take max)
                            need = max(lvl[(h, g)] for g in range(gi_max + 1))
                            m = max(m, need)
                    cls.append(m)
                qcls[(s, t, h)] = cls

    # ---- build calls: per super-group, per (h, s) segment, class-sorted ----
    sub_src = []            # (s, t, h, j) per subchunk, final order
    sub_tile = []
    calls = []              # dict: s, h, sub0, n_sub, q, sg
    for sgi, tiles in enumerate(C.SGS):
        sg_calls = []
        for h in range(2):
            for s in range(S):
                ordered = []
                for t in tiles:
                    for j in range(int(L[s][t][h]) // 128):
                        ordered.append((qcls[(s, t, h)][j], t, j))
                ordered.sort()
                o = 0
                while o < len(ordered):
                    take = min(C.GCH, len(ordered) - o)
                    chunk = ordered[o:o + take]
                    sg_calls.append(dict(
                        s=s, h=h, sg=sgi, subs=chunk,
                        q=max(cl for cl, _, _ in chunk)))
                    o += take
        sg_calls.sort(key=lambda cl: cl["q"])
        for cl in sg_calls:
            cl["sub0"] = len(sub_src)
            cl["n_sub"] = len(cl["subs"])
            for (_, t, j) in cl["subs"]:
                sub_src.append((cl["s"], t, cl["h"], j))
                sub_tile.append(t)
            calls.append(cl)
    NSUB = len(sub_src)
    TOT = NSUB * 128
    assert TOT == int(L.sum())

    # Per-subchunk max real count over cores (for trailing-pad skip)
    def _nreal(sc):
        s, t, h, j = sc
        m = 0
        for c in range(M):
            m = max(m, min(128, max(0, len(raw[c][s][t][h][0]) - j * 128)))
        return m

    # within each call, move the subchunk with most skippable trailing pads
    # to the end; record the call's real (non-skipped) index count
    for cl in calls:
        a, b = cl["sub0"], cl["sub0"] + cl["n_sub"]
        pads = [128 - _nreal(sub_src[i]) for i in range(a, b)]
        kbest = int(np.argmax(pads))
        sub_src[a + kbest:b] = sub_src[a + kbest + 1:b] + [sub_src[a + kbest]]
        st = list(sub_tile[a:b])
        st[kbest:] = st[kbest + 1:] + [st[kbest]]
        sub_tile[a:b] = st
        cl["nireg"] = cl["n_sub"] * 128 - pads[kbest]

    # last subchunk per tile and the call index containing it
    last_sub = np.full(C.NT, -1, dtype=np.int64)
    for i, t in enumerate(sub_tile):
        last_sub[t] = i
    assert (last_sub >= 0).all()
    sub_call = np.zeros(NSUB, dtype=np.int64)
    for ci, cl in enumerate(calls):
        sub_call[cl["sub0"]: cl["sub0"] + cl["n_sub"]] = ci
    k_last = sub_call[last_sub]    # call index of each tile's last subchunk

    sched = dict(L=L, calls=calls, sub_tile=sub_tile, TOT=TOT,
                 NSUB=NSUB, last_sub=last_sub, k_last=k_last,
                 sub_call=sub_call)

    # per-core flattened arrays (slot layout follows sub_src permutation)
    per_core = []
    for c in range(M):
        idx = np.zeros(TOT, dtype=np.int16)
        dl = np.zeros(TOT, dtype=np.int64)
        vl = np.zeros(TOT, dtype=np.float32)
        for i, (s, t, h, j) in enumerate(sub_src):
            i16, d16, v32 = raw[c][s][t][h]
            a, b = j * 128, min((j + 1) * 128, len(i16))
            n = max(0, b - a)
            o = i * 128
            if n > 0:
                idx[o:o + n] = i16[a:b]
                dl[o:o + n] = d16[a:b]
                vl[o:o + n] = v32[a:b]
        for cl in calls:
            oe = (cl["sub0"] + cl["n_sub"]) * 128
            skip = cl["n_sub"] * 128 - cl["nireg"]
            if skip:
                idx[oe - skip:oe] = -1
        eidx = np.tile(idx.reshape(TOT // 16, 16).T, (8, 1))      # [128, TOT/16]
        # per-subchunk (dstloc, val) sidebands, CALL-ALIGNED columns
        NCALL = len(calls)
        dloc = np.zeros((128, NCALL * C.GCH), dtype=np.float32)
        vals = np.zeros((128, NCALL * C.GCH), dtype=np.float32)
        dl2 = dl.reshape(TOT // 128, 128).T.astype(np.float32)
        vl2 = vl.reshape(TOT // 128, 128).T
        for b, cl in enumerate(calls):
            a0, ns = cl["sub0"], cl["n_sub"]
            dloc[:, b * C.GCH:b * C.GCH + ns] = dl2[:, a0:a0 + ns]
            vals[:, b * C.GCH:b * C.GCH + ns] = vl2[:, a0:a0 + ns]
        per_core.append(dict(eidx=eidx, dloc=dloc, vals=vals))
    return sched, per_core


# ----------------------------------------------------------- graph build ---

def _build_graph(C, pa, sched):
    import concourse.bass as bass
    import concourse.bacc as bacc
    import concourse.mybir as mybir
    from concourse.library_config import mlp
    from contextlib import ExitStack

    f32, bf16, i16 = mybir.dt.float32, mybir.dt.bfloat16, mybir.dt.int16
    S = C.S
    SW = S * 128                       # pre2 row width
    TOT = sched["TOT"]
    T16 = TOT // 16
    calls = sched["calls"]
    sub_tile = sched["sub_tile"]
    last_sub = set(sched["last_sub"].tolist())
    k_last = sched["k_last"]
    NCALL = len(calls)
    GCH, RB, RBTB = C.GCH, C.RB, C.RBTB
    TPAD = NCALL * GCH

    sup_order = pa["sup_order"]
    em = pa["em"]
    tile_of_em = pa["tile_of_em"]
    grp_order = pa["grp_order"]
    grank = pa["rank"]
    per_lvl = C.NG // C.NLV

    # PSUM region per dst tile: super-group sg, local tl -> bank 2+tl//4,
    # col (tl%4)*128
    def _region(t):
        for sgi, tiles in enumerate(C.SGS):
            if t in tiles:
                tl = t - tiles[0]
                return sgi, 2 + tl // 4, (tl % 4) * 128
        raise AssertionError(t)

    # ACT relu order: per sg, banks ascending, 4 tiles each (em order of
    # regions). rlu counter increments once per tile after relu (+rezero).
    act_order = []             # (sg, bank, col, t, rezero)
    for sgi, tiles in enumerate(C.SGS):
        by_bank = {}
        for t in tiles:
            _, b, col = _region(t)
            by_bank.setdefault(b, []).append((col, t))
        for b in sorted(by_bank):
            for col, t in sorted(by_bank[b]):
                act_order.append((sgi, b, col, t, sgi + 1 < len(C.SGS)))
    # rlu threshold for tensor: before FIRST touch of bank b in sg>=1, wait
    # until all of sg-1's tiles on bank b are relu'd+rezeroed. rlu counts
    # REZERO completions only (in act_order order; non-rz tiles don't inc).
    rlu_thresh = {}
    nrz = 0
    for (sgi, b, col, t, rz) in act_order:
        if rz:
            nrz += 1
            rlu_thresh[(sgi + 1, b)] = nrz
    # per-(sg,bank) peB threshold for ACT: max k_last over the bank's tiles
    peb_thresh = {}
    for (sgi, b, col, t, rz) in act_order:
        key = (sgi, b)
        peb_thresh[key] = max(peb_thresh.get(key, 0), int(k_last[t]) + 1)

    nc = bacc.Bacc("TRN2", num_swdge_queues=C.NQ)
    hT = nc.declare_dram_parameter("hT", [128, C.NP2], bf16, isOutput=False)
    xT = nc.declare_dram_parameter("xT", [128, C.NP2], bf16, isOutput=False)
    v0d = nc.declare_dram_parameter("v0", [128, SW], bf16, isOutput=False)
    v1d = nc.declare_dram_parameter("v1", [128, SW], bf16, isOutput=False)
    eidxd = nc.declare_dram_parameter("eidx", [128, T16], i16, isOutput=False)
    dlocd = nc.declare_dram_parameter("dloc", [128, TPAD], f32, isOutput=False)
    valsd = nc.declare_dram_parameter("vals", [128, TPAD], f32, isOutput=False)
    iotad = nc.declare_dram_parameter("iota", [128, 128], bf16, isOutput=False)
    outd = nc.declare_dram_parameter("out", [C.NPAD, 128], f32, isOutput=True)
    pre2 = nc.dram_tensor("pre2", [C.NP2, SW], bf16)

    with ExitStack() as ctx:
        ec = ctx.enter_context
        # SBUF
        hx_sb = [[ec(nc.sbuf_tensor(f"hx{b}{k}", [128, C.SUP], bf16))
                  for k in range(2)] for b in range(2)]
        v_sb = [ec(nc.sbuf_tensor(f"v{k}_sb", [128, SW], bf16)) for k in range(2)]
        eidx_sb = ec(nc.sbuf_tensor("eidx_sb", [128, T16], i16))
        pout_sb = [[ec(nc.sbuf_tensor(f"pout{h}{b}", [128, C.GRP, SW], bf16))
                    for b in range(2)] for h in range(2)]
        msg_sb = [ec(nc.sbuf_tensor(f"msg{b}", [128, GCH, 128], bf16))
                  for b in range(RB)]
        tb_sb = ec(nc.sbuf_tensor("tb_sb", [128, RBTB * GCH, 128], bf16))
        dloc_sb = ec(nc.sbuf_tensor("dloc_sb", [128, TPAD], f32))
        vals_sb = ec(nc.sbuf_tensor("vals_sb", [128, TPAD], f32))
        iota_sb = ec(nc.sbuf_tensor("iota_sb", [128, 128], bf16))
        zero_sb = ec(nc.sbuf_tensor("zero_sb", [128, 128], f32))
        outb_sb = [ec(nc.sbuf_tensor(f"ob{b}", [128, 128], f32)) for b in range(4)]
        # PSUM: 8 full banks; 0/1 phase A, 2..7 hold 4 dst regions each
        psum = [ec(nc.psum_tensor(f"ps{b}", [128, 512], f32)) for b in range(8)]
        # semaphores
        in_sem = ec(nc.semaphore("in_sem"))
        hxs = [ec(nc.semaphore(f"hxs{i}")) for i in range(2)]
        p2wd = ec(nc.semaphore("p2wd"))
        gths = [ec(nc.semaphore(f"gths{i}")) for i in range(RB)]
        dve_c = ec(nc.semaphore("dve_c"))
        outws = [ec(nc.semaphore(f"outws{i}")) for i in range(4)]
        mz = ec(nc.semaphore("mz"))
        zps = ec(nc.semaphore("zps"))
        rel = ec(nc.semaphore("rel"))
        mmA = ec(nc.semaphore("mmA"))
        cpy = ec(nc.semaphore("cpy"))
        peB = ec(nc.semaphore("peB"))
        rlu = ec(nc.semaphore("rlu"))

        with nc.Block() as block:

            @block.sync
            def _(sync):
                sync.dma_start(out=v_sb[0][:], in_=v0d[:]).then_inc(in_sem, 16)
                sync.dma_start(out=v_sb[1][:], in_=v1d[:]).then_inc(in_sem, 16)
                sync.dma_start(out=eidx_sb[:], in_=eidxd[:]).then_inc(in_sem, 16)
                sync.dma_start(out=dloc_sb[:], in_=dlocd[:]).then_inc(in_sem, 16)
                sync.dma_start(out=vals_sb[:], in_=valsd[:]).then_inc(in_sem, 16)
                sync.dma_start(out=iota_sb[:], in_=iotad[:]).then_inc(in_sem, 16)
                for pos in range(C.NSUP):
                    if pos >= 2:
                        sync.wait_ge(mmA, C.T20 * (pos - 1))
                    sup = sup_order[pos]
                    sl = slice(sup * C.SUP, (sup + 1) * C.SUP)
                    sync.dma_start(out=hx_sb[pos % 2][0][:],
                                   in_=hT[:, sl]).then_inc(hxs[pos % 2], 16)
                    sync.dma_start(out=hx_sb[pos % 2][1][:],
                                   in_=xT[:, sl]).then_inc(hxs[pos % 2], 16)

            # interleave schedule: which early calls run between phase-A
            # supers (level-gated so their pre2 data exists)
            # level lv ready once group rank per_lvl*(lv+1)-1 is DMA'd; use
            # its em_last + margin as the earliest super position.
            em_need = [pa["em_last"][grp_order[per_lvl * (lv + 1) - 1]]
                       for lv in range(C.NLV)]
            ILV_CAP = 120
            ILV_TAKE = 6
            ilv_sched = {}
            ci = 0
            for pos in range(C.NSUP):
                avail = -1
                for lv in range(C.NLV):
                    if em_need[lv] + 12 <= (pos + 1) * C.T20:
                        avail = lv
                take = []
                while (ci < min(ILV_CAP, NCALL) and len(take) < ILV_TAKE
                       and calls[ci]["q"] <= avail):
                    take.append(ci)
                    ci += 1
                if take:
                    ilv_sched[pos] = take
            n_ilv = ci

            @block.tensor
            def _(tensor):
                state = dict(init=False, bank_seen=set())

                def _emit_call(ci):
                    cl = calls[ci]
                    if not state["init"]:
                        tensor.wait_ge(mz, RB)
                        tensor.wait_ge(zps, 7)
                        state["init"] = True
                    tensor.wait_ge(gths[ci % RB], 16 * (ci // RB + 1))
                    tensor.wait_ge(dve_c, ci + 1)
                    for k in range(cl["n_sub"]):
                        i = cl["sub0"] + k
                        t = int(sub_tile[i])
                        sgi, bank, col = _region(t)
                        if sgi >= 1 and (sgi, bank) not in state["bank_seen"]:
                            state["bank_seen"].add((sgi, bank))
                            tensor.wait_ge(rlu, rlu_thresh[(sgi, bank)])
                        mm = tensor.matmul(
                            psum[bank][:, col:col + 128],
                            tb_sb[:, (ci % RBTB) * GCH + k, :],
                            msg_sb[ci % RB][:, k, :],
                            start=False, stop=(i in last_sub),
                            skip_group_check=True,
                        )
                        if k == cl["n_sub"] - 1:
                            mm.then_inc(peB)

                # ---- phase A (banks 0/1) with interleaved early calls ----
                tensor.wait_ge(in_sem, 96)
                for pos in range(C.NSUP):
                    tensor.wait_ge(hxs[pos % 2], 32 * (pos // 2 + 1))
                    for t20 in range(C.T20):
                        e = pos * C.T20 + t20
                        if e >= 2:
                            tensor.wait_ge(cpy, e - 1)
                        sl = slice(t20 * 128, (t20 + 1) * 128)
                        tensor.matmul(psum[e % 2][:, :], hx_sb[pos % 2][0][:, sl],
                                      v_sb[0][:], start=True, stop=False,
                                      skip_group_check=True)
                        tensor.matmul(psum[e % 2][:, :], hx_sb[pos % 2][1][:, sl],
                                      v_sb[1][:], start=False, stop=True,
                                      skip_group_check=True).then_inc(mmA)
                    for cix in ilv_sched.get(pos, []):
                        _emit_call(cix)
                # ---- phase B ----
                for cix in range(n_ilv, NCALL):
                    _emit_call(cix)

            @block.scalar
            def _(scalar):
                Copy = mybir.ActivationFunctionType.Copy
                Relu = mybir.ActivationFunctionType.Relu
                # ---- phase A: psum -> bf16 pout (strict emission order) ----
                # one DMA per completed 8-tile group (within-half-contiguous);
                # group DMAs complete in issue order (same HWDGE ring).
                for e in range(C.NTA):
                    j = int(tile_of_em[e])
                    h, jh = j // C.NTH, j % C.NTH
                    gi, q8 = jh // C.GRP, jh % C.GRP
                    if q8 == 0 and gi >= 2:
                        scalar.wait_ge(p2wd, 16 * (grank[(h, gi - 2)] + 1))
                    scalar.wait_ge(mmA, e + 1)
                    scalar.activation(pout_sb[h][gi % 2][:, q8, :],
                                      psum[e % 2][:, :], Copy).then_inc(cpy)
                    if q8 == C.GRP - 1:
                        scalar.wait_ge(cpy, e + 1)
                        ph = pre2[h * C.HALF:(h + 1) * C.HALF, :].rearrange(
                            "(p j) c -> p j c", p=128)
                        scalar.dma_start(
                            out=ph[:, gi * C.GRP:(gi + 1) * C.GRP, :],
                            in_=pout_sb[h][gi % 2][:, :, :]).then_inc(p2wd, 16)
                # ---- phase B: relu psum regions -> out, rezero regions ----
                scalar.wait_ge(zps, 7)     # zero_sb ready
                r = 0
                nouts = [0, 0, 0, 0]
                done_bank = set()
                for (sgi, bank, col, t, rz) in act_order:
                    if (sgi, bank) not in done_bank:
                        done_bank.add((sgi, bank))
                        scalar.wait_ge(peB, peb_thresh[(sgi, bank)])
                    if r >= 4:
                        scalar.wait_ge(outws[r % 4], 16 * (r // 4))
                    scalar.activation(outb_sb[r % 4][:],
                                      psum[bank][:, col:col + 128],
                                      Relu).then_inc(rel)
                    scalar.wait_ge(rel, r + 1)
                    if rz:
                        scalar.activation(psum[bank][:, col:col + 128],
                                          zero_sb[:], Copy).then_inc(rlu)
                    scalar.dma_start(out=outd[t * 128:(t + 1) * 128, :],
                                     in_=outb_sb[r % 4][:]).then_inc(outws[r % 4], 16)
                    nouts[r % 4] += 1
                    r += 1
                for q in range(4):
                    if nouts[q]:
                        scalar.wait_ge(outws[q], 16 * nouts[q])

            @block.vector
            def _(vector):
                ieq, mul = mybir.AluOpType.is_equal, mybir.AluOpType.mult
                # zero-init phase-B psum banks (values; matmuls use
                # accumulate-or-overwrite so either bit state is correct)
                vector.memset(zero_sb[:], 0.0).then_inc(zps)
                for b in range(2, 8):
                    vector.memset(psum[b][:, :], 0.0).then_inc(zps)
                vector.wait_ge(in_sem, 96)
                for b in range(NCALL):
                    ns = calls[b]["n_sub"]
                    if b >= RBTB:
                        vector.wait_ge(peB, b - RBTB + 1)
                    s0 = (b % RBTB) * GCH
                    for k in range(ns):
                        op = vector.tensor_scalar(
                            tb_sb[:, s0 + k, :], iota_sb[:, :],
                            dloc_sb[:, b * GCH + k:b * GCH + k + 1],
                            vals_sb[:, b * GCH + k:b * GCH + k + 1],
                            ieq, mul)
                        if k == ns - 1:
                            op.then_inc(dve_c)

            @block.gpsimd
            def _(gpsimd):
                gpsimd.load_library(mlp)
                gpsimd.wait_ge(in_sem, 96)
                for b in range(RB):
                    gpsimd.memzero(msg_sb[b][:]).then_inc(mz)
                gpsimd.wait_ge(mz, RB)
                nreg = nc.alloc_register(mybir.EngineType.Pool, "nidx")
                cur_lvl = -1
                for ci, cl in enumerate(calls):
                    s, h = cl["s"], cl["h"]
                    if cl["q"] > cur_lvl:
                        cur_lvl = cl["q"]
                        gpsimd.wait_ge(p2wd, 16 * per_lvl * (cur_lvl + 1))
                    if ci >= RB:
                        gpsimd.wait_ge(peB, ci - RB + 1)
                    n_idx = cl["n_sub"] * 128
                    src_ap = pre2[h * C.HALF:(h + 1) * C.HALF,
                                  s * 128:(s + 1) * 128]
                    o16 = cl["sub0"] * 8  # *128/16
                    gpsimd.reg_mov(nreg, cl["nireg"])
                    gpsimd.dma_gather(
                        msg_sb[ci % RB][:, 0:cl["n_sub"], :],
                        src_ap,
                        eidx_sb[:, o16:o16 + n_idx // 16],
                        n_idx, nreg, 128, elem_step=SW,
                        single_packet=False,
                        queue_num=ci % C.NQ,
                    ).then_inc(gths[ci % RB], 16)

    nc.compile()
    return nc


# ------------------------------------------------------------ entrypoint ---

_CACHE = {}


def _get_graph(C, sched_key, pa, sched):
    if sched_key not in _CACHE:
        _CACHE[sched_key] = _build_graph(C, pa, sched)
    return _CACHE[sched_key]


def _host_prep(h, x, W, inp_W, mix_w, inp_mix_w, edge_val, edge_src, edge_dst, C):
    v0, v1 = _fold_weights(np.asarray(W), np.asarray(inp_W),
                           np.asarray(mix_w), np.asarray(inp_mix_w), C)
    hTf = np.zeros((128, C.NP2), dtype=BF16)
    xTf = np.zeros((128, C.NP2), dtype=BF16)
    hTf[:, :C.N] = np.asarray(h, dtype=np.float32).T.astype(BF16)
    xTf[:, :C.N] = np.asarray(x, dtype=np.float32).T.astype(BF16)
    pa = _phase_a_meta(C)
    sched, per_core = _prep_edges(np.asarray(edge_src), np.asarray(edge_dst),
                                  np.asarray(edge_val, dtype=np.float32), C, pa)
    iota = np.ascontiguousarray(
        np.broadcast_to(np.arange(128, dtype=np.float32)[None, :],
                        (128, 128))).astype(BF16)
    in_maps = []
    for c in range(C.M):
        in_maps.append(dict(
            hT=np.ascontiguousarray(hTf), xT=np.ascontiguousarray(xTf),
            v0=v0, v1=v1,
            eidx=np.ascontiguousarray(per_core[c]["eidx"]),
            dloc=per_core[c]["dloc"], vals=per_core[c]["vals"],
            iota=iota,
        ))
    return pa, sched, in_maps


def kernel(h, x, W, inp_W, mix_w, inp_mix_w, edge_val, edge_src, edge_dst,
           _cfg=None, _trace=False):
    C = _cfg or FULL
    pa, sched, in_maps = _host_prep(h, x, W, inp_W, mix_w, inp_mix_w,
                                    edge_val, edge_src, edge_dst, C)
    key = (C.N, C.E, sched["TOT"], sched["NSUB"])
    nc = _get_graph(C, key, pa, sched)

    from concourse.bass_utils import run_bass_kernel_spmd
    res = run_bass_kernel_spmd(nc, in_maps, core_ids=list(range(C.M)),
                               trace=_trace)
    out = np.empty((C.N, 128), dtype=np.float32)
    for c in range(C.M):
        out[c * C.NSH:(c + 1) * C.NSH] = res.results[c]["out"][:C.NSH]
    kernel._last_exec_ns = res.exec_time_ns
    return out


# revision 12
# speedup vs baseline: 1.8275x; 1.8275x over previous
"""Distributed Trainium2 kernel for AdaptiveGraphRecursiveConvolution.

Math (reference):
    out = relu( sum_g mix_w[g] * sum_k A_{gk} @ (h @ W[g,k])
              + sum_g inp_mix_w[g] * sum_k A_{gk} @ (x @ inp_W[g,k]) )

Folding the scalar mixing weights into the dense weights and merging the
h/x paths gives, with S = G*K edge sets and V_s = [mix_w*W_s ; inp_mix*inp_W_s]:
    out = relu( sum_s A_s @ (hx @ V_s) ),  hx = [h | x]  (N x 2F)

Device strategy (8 NeuronCores, SPMD single graph, per-core data):
  - dst-nodes sharded: core c owns rows [c*NSH, (c+1)*NSH).
  - Phase A (replicated): pre2 = hx @ [V_0..V_3]  ([NP2, S*128] bf16) via PE,
    streamed to private DRAM. Node supertiles are emitted with the two
    int16-index halves INTERLEAVED so both halves become gatherable early.
  - Phase B: per edge-set SpMM. Edges (pre-sharded/sorted/padded on host) are
    gathered per-edge from pre2 via SWDGE dma_gather (bf16 rows), and
    scatter-added into PSUM dst regions via one-hot matmuls on PE:
        psum[region(dst_tile)] += T_chunk^T @ msg_chunk
    where T[e, j] = val_e * (dst_loc_e == j) is built on DVE with one fused
    tensor_scalar (is_equal then mult) per 128-edge subchunk.
  - PSUM packing: 4 dst tiles per bank (regions), matmuls never use
    start=True on phase-B banks; regions are value-zeroed before reuse so
    accumulate-or-overwrite is correct either way. Tiles are grouped in
    super-groups of 24 (6 banks x 4 regions); calls within a super-group are
    sorted by pre2-readiness level so phase B streams during phase A.
  - relu on ACT from PSUM, DMA out. Host concatenates the 8 shards.
"""

import os
import sys
import time

import numpy as np

sys.path.insert(0, "/opt/trn_rl_repo")
sys.path.insert(0, "/root/.axon_site/_ro/trn_rl_repo")

import ml_dtypes  # noqa: E402

BF16 = ml_dtypes.bfloat16

_ONEHOT = None


def _register_onehot_op():
    """Custom DVE op: out[p, t] = (t == in1[p, t]) * in0[p, t], with t the
    element position along the free dims. With in1 = dlocg (128*k + dstloc,
    per-subchunk column broadcast along the last dim) and in0 = vals
    broadcast, one instruction builds the scaled one-hot T block for a whole
    gather call."""
    global _ONEHOT
    if _ONEHOT is not None:
        return _ONEHOT
    from concourse import dve_ops
    from concourse.dve_spec import Spec, Src0, Src1, Idx, eq, lower
    from concourse.dve_uop import DveOpSpec

    NAME = "ONEHOT_VAL_ANT"
    if NAME in dve_ops._SUB_OPCODE_FOR_NAME:
        _ONEHOT = next(o for o in dve_ops.OPS if o.name == NAME)
        return _ONEHOT

    def _ref(in0, in1, s0, s1, imm2):
        P = in1.shape[0]
        f1 = np.asarray(in1, np.float32).reshape(P, -1)
        f0 = np.asarray(in0, np.float32).reshape(P, -1)
        idx = np.arange(f1.shape[1], dtype=np.float32)[None, :]
        return (idx == f1).astype(np.float32) * f0

    spec = Spec(body=eq(Idx, Src1) * Src0, reference=_ref)
    opc = max(dve_ops._SUB_OPCODE_FOR_NAME.values()) + 1
    assert opc < 0x20
    shas = {}
    for ver in ("v3", "v4"):
        s = DveOpSpec(name=NAME, opcode=opc, uops=lower(spec, ver=ver),
                      rd1_en=True)
        shas[ver] = s.sha(ver)
    op = dve_ops.DveOp(NAME, spec, subdim=False, uops_sha=shas)
    dve_ops.OPS.append(op)
    dve_ops.CUSTOM_DVE_SPECS[NAME] = spec
    dve_ops._SUB_OPCODE_FOR_NAME[NAME] = opc
    _ONEHOT = op
    return op


# ---------------------------------------------------------------- config ---

class Cfg:
    def __init__(self, N, E, S=4, M=8, HALF=None, SUP=None, GCH=8, SGT=24,
                 RB=12, RBTB=12, NQ=4, NLV=8):
        self.N, self.E, self.S, self.M = N, E, S, M
        self.F2 = 256          # hx feature dim (2*128)
        self.O = 128
        self.NSH = N // M      # dst rows owned per core
        self.NPAD = _ru(self.NSH, 128)
        self.NT = self.NPAD // 128          # dst tiles per core
        self.SGT = SGT                      # tiles per super-group (<= 24)
        self.SGS = [list(range(a, min(a + SGT, self.NT)))
                    for a in range(0, self.NT, SGT)]
        self.HALF = HALF if HALF else _ru(-(-N // 2), 128)
        assert self.HALF <= 32767
        self.NP2 = 2 * self.HALF            # padded node rows in pre2
        assert N <= self.NP2
        self.SUP = SUP if SUP else 2560     # nodes per phase-A supertile
        assert self.SUP % 128 == 0 and self.NP2 % self.SUP == 0
        self.NSUP = self.NP2 // self.SUP
        assert self.NSUP % 2 == 0
        self.T20 = self.SUP // 128
        self.NTA = self.NP2 // 128          # phase-A node tiles
        self.GCH = GCH                      # max subchunks (128 edges) per gather
        self.RB = RB                        # msg ring depth (calls)
        self.RBTB = RBTB                    # T ring depth (calls)
        self.NQ = NQ                        # SWDGE queues
        self.NLV = NLV                      # pre2 readiness levels
        self.GRP = 8                        # tiles per pre2 write group
        self.NTH = self.HALF // 128         # node tiles per half
        self.NGH = self.NTH // self.GRP     # write groups per half
        self.NG = 2 * self.NGH
        assert self.NG % NLV == 0


def _ru(x, m):
    return (x + m - 1) // m * m


FULL = Cfg(N=40000, E=640000, HALF=20480, SUP=2560)


# ------------------------------------------------------------- host prep ---

def _fold_weights(W, inp_W, mix_w, inp_mix_w, C):
    """Return v0, v1: [128, S*128] bf16 (h-path and x-path stationary weights)."""
    G, K = W.shape[0], W.shape[1]
    S = G * K
    Wm = (W.astype(np.float64) * mix_w.astype(np.float64)[:, None, None, None])
    Im = (inp_W.astype(np.float64) * inp_mix_w.astype(np.float64)[:, None, None, None])
    v0 = Wm.reshape(S, 128, 128).transpose(1, 0, 2).reshape(128, S * 128)
    v1 = Im.reshape(S, 128, 128).transpose(1, 0, 2).reshape(128, S * 128)
    return v0.astype(BF16), v1.astype(BF16)


def _phase_a_meta(C):
    """Emission order of phase-A supertiles/groups with interleaved halves."""
    H = C.NSUP // 2
    sup_order = []
    for k in range(H):
        sup_order += [k, H + k]
    # em[j]: emission index of phase-A node tile j (j = NP2-row-block index)
    em = np.zeros(C.NTA, dtype=np.int64)
    for pos, sup in enumerate(sup_order):
        for t20 in range(C.T20):
            em[sup * C.T20 + t20] = pos * C.T20 + t20
    tile_of_em = np.zeros(C.NTA, dtype=np.int64)
    tile_of_em[em] = np.arange(C.NTA)
    # groups (h, gi): tiles jh = gi*GRP .. +GRP-1 of half h
    groups = [(h, gi) for h in range(2) for gi in range(C.NGH)]
    em_last = {}
    for (h, gi) in groups:
        tiles = [h * C.NTH + gi * C.GRP + q for q in range(C.GRP)]
        em_last[(h, gi)] = int(max(em[t] for t in tiles))
    grp_order = sorted(groups, key=lambda g: em_last[g])
    rank = {g: i for i, g in enumerate(grp_order)}
    per_lvl = C.NG // C.NLV
    lvl = {g: rank[g] // per_lvl for g in groups}
    return dict(sup_order=sup_order, em=em, tile_of_em=tile_of_em,
                grp_order=grp_order, rank=rank, lvl=lvl, em_last=em_last)


def _prep_edges(edge_src, edge_dst, edge_val, C, pa):
    """Shard/sort/pad edges. Returns (sched, per_core)."""
    S, E, M = C.S, C.E, C.M
    src = edge_src.reshape(S, E).astype(np.int64)
    dst = edge_dst.reshape(S, E).astype(np.int64)
    val = edge_val.reshape(S, E).astype(np.float32)

    NTH = C.NTH
    lvl = pa["lvl"]
    # raw[c][s][t][h] = (idx16, dstloc, val); pre2 rows are PERMUTED within
    # each half: node (p=sl%128, j=sl//128) is stored at row p*NTH + j.
    raw = [[[[None, None] for _ in range(C.NT)] for _ in range(S)] for _ in range(M)]
    cnt = np.zeros((M, S, C.NT, 2), dtype=np.int64)
    for s in range(S):
        core_of = dst[s] // C.NSH
        for c in range(M):
            sel = np.nonzero(core_of == c)[0]
            d = dst[s][sel] - c * C.NSH
            t = d // 128
            h = (src[s][sel] >= C.HALF).astype(np.int64)
            # secondary sort by src so low-src subchunks can gather early
            key = (t * 2 + h) * (2 * C.HALF) + src[s][sel]
            order = np.argsort(key, kind="stable")
            sel, d, t, h = sel[order], d[order], t[order], h[order]
            key = key[order]
            bounds = np.searchsorted(key, np.arange(C.NT * 2 + 1) * (2 * C.HALF))
            for ti in range(C.NT):
                for hi in range(2):
                    a, b = bounds[ti * 2 + hi], bounds[ti * 2 + hi + 1]
                    ss = sel[a:b]
                    sl = src[s][ss] - hi * C.HALF
                    raw[c][s][ti][hi] = (
                        ((sl % 128) * NTH + sl // 128).astype(np.int16),
                        (d[a:b] - ti * 128).astype(np.int16),
                        val[s][ss],
                    )
                    cnt[c, s, ti, hi] = b - a

    # common padded lengths
    L = np.maximum(cnt.max(axis=0), 1)
    L = ((L + 127) // 128 * 128)  # [S, NT, 2]

    # Per-(s,t,h) per-subchunk readiness class: max over cores of the pre2
    # write-group LEVEL of the rows the subchunk gathers.
    qcls = {}
    for s in range(S):
        for t in range(C.NT):
            for h in range(2):
                nsub = int(L[s][t][h]) // 128
                cls = []
                for j in range(nsub):
                    m = 0
                    for c in range(M):
                        seg_i = raw[c][s][t][h][0][j * 128:(j + 1) * 128]
                        if len(seg_i):
                            gi_max = int((seg_i % NTH).max()) // C.GRP
                            # rows in this subchunk span groups up to gi_max;
                            # level needed = max level among groups 0..gi_max
                            # (sorted-by-src makes gi_max the binding one, but
                            # levels are not monotone in gi -> take max)
                            need = max(lvl[(h, g)] for g in range(gi_max + 1))
                            m = max(m, need)
                    cls.append(m)
                qcls[(s, t, h)] = cls

    # ---- build calls: per super-group, per (h, s) segment, class-sorted ----
    sub_src = []            # (s, t, h, j) per subchunk, final order
    sub_tile = []
    calls = []              # dict: s, h, sub0, n_sub, q, sg
    for sgi, tiles in enumerate(C.SGS):
        sg_calls = []
        for h in range(2):
            for s in range(S):
                ordered = []
                for t in tiles:
                    for j in range(int(L[s][t][h]) // 128):
                        ordered.append((qcls[(s, t, h)][j], t, j))
                ordered.sort()
                o = 0
                while o < len(ordered):
                    take = min(C.GCH, len(ordered) - o)
                    chunk = ordered[o:o + take]
                    sg_calls.append(dict(
                        s=s, h=h, sg=sgi, subs=chunk,
                        q=max(cl for cl, _, _ in chunk)))
                    o += take
        sg_calls.sort(key=lambda cl: cl["q"])
        for cl in sg_calls:
            cl["sub0"] = len(sub_src)
            cl["n_sub"] = len(cl["subs"])
            for (_, t, j) in cl["subs"]:
                sub_src.append((cl["s"], t, cl["h"], j))
                sub_tile.append(t)
            calls.append(cl)
    NSUB = len(sub_src)
    TOT = NSUB * 128
    assert TOT == int(L.sum())

    # Per-subchunk max real count over cores (for trailing-pad skip)
    def _nreal(sc):
        s, t, h, j = sc
        m = 0
        for c in range(M):
            m = max(m, min(128, max(0, len(raw[c][s][t][h][0]) - j * 128)))
        return m

    # within each call, move the subchunk with most skippable trailing pads
    # to the end; record the call's real (non-skipped) index count
    for cl in calls:
        a, b = cl["sub0"], cl["sub0"] + cl["n_sub"]
        pads = [128 - _nreal(sub_src[i]) for i in range(a, b)]
        kbest = int(np.argmax(pads))
        sub_src[a + kbest:b] = sub_src[a + kbest + 1:b] + [sub_src[a + kbest]]
        st = list(sub_tile[a:b])
        st[kbest:] = st[kbest + 1:] + [st[kbest]]
        sub_tile[a:b] = st
        cl["nireg"] = cl["n_sub"] * 128 - pads[kbest]

    # last subchunk per tile and the call index containing it
    last_sub = np.full(C.NT, -1, dtype=np.int64)
    for i, t in enumerate(sub_tile):
        last_sub[t] = i
    assert (last_sub >= 0).all()
    sub_call = np.zeros(NSUB, dtype=np.int64)
    for ci, cl in enumerate(calls):
        sub_call[cl["sub0"]: cl["sub0"] + cl["n_sub"]] = ci
    k_last = sub_call[last_sub]    # call index of each tile's last subchunk

    sched = dict(L=L, calls=calls, sub_tile=sub_tile, TOT=TOT,
                 NSUB=NSUB, last_sub=last_sub, k_last=k_last,
                 sub_call=sub_call)

    # per-core flattened arrays (slot layout follows sub_src permutation)
    per_core = []
    for c in range(M):
        idx = np.zeros(TOT, dtype=np.int16)
        dl = np.zeros(TOT, dtype=np.int64)
        vl = np.zeros(TOT, dtype=np.float32)
        for i, (s, t, h, j) in enumerate(sub_src):
            i16, d16, v32 = raw[c][s][t][h]
            a, b = j * 128, min((j + 1) * 128, len(i16))
            n = max(0, b - a)
            o = i * 128
            if n > 0:
                idx[o:o + n] = i16[a:b]
                dl[o:o + n] = d16[a:b]
                vl[o:o + n] = v32[a:b]
        for cl in calls:
            oe = (cl["sub0"] + cl["n_sub"]) * 128
            skip = cl["n_sub"] * 128 - cl["nireg"]
            if skip:
                idx[oe - skip:oe] = -1
        eidx = np.tile(idx.reshape(TOT // 16, 16).T, (8, 1))      # [128, TOT/16]
        # per-subchunk (dstloc, val) sidebands, CALL-ALIGNED columns
        NCALL = len(calls)
        dloc = np.zeros((128, NCALL * C.GCH), dtype=np.float32)
        vals = np.zeros((128, NCALL * C.GCH), dtype=np.float32)
        dl2 = dl.reshape(TOT // 128, 128).T.astype(np.float32)
        vl2 = vl.reshape(TOT // 128, 128).T
        koff = np.arange(C.GCH, dtype=np.float32) * 128.0
        for b, cl in enumerate(calls):
            a0, ns = cl["sub0"], cl["n_sub"]
            dloc[:, b * C.GCH:b * C.GCH + ns] = (dl2[:, a0:a0 + ns]
                                                 + koff[None, :ns])
            vals[:, b * C.GCH:b * C.GCH + ns] = vl2[:, a0:a0 + ns]
        per_core.append(dict(eidx=eidx, dloc=dloc, vals=vals))
    return sched, per_core


# ----------------------------------------------------------- graph build ---

def _build_graph(C, pa, sched):
    import concourse.bass as bass
    import concourse.bacc as bacc
    import concourse.mybir as mybir
    from concourse.library_config import mlp
    from contextlib import ExitStack

    f32, bf16, i16 = mybir.dt.float32, mybir.dt.bfloat16, mybir.dt.int16
    S = C.S
    SW = S * 128                       # pre2 row width
    TOT = sched["TOT"]
    T16 = TOT // 16
    calls = sched["calls"]
    sub_tile = sched["sub_tile"]
    last_sub = set(sched["last_sub"].tolist())
    k_last = sched["k_last"]
    NCALL = len(calls)
    GCH, RB, RBTB = C.GCH, C.RB, C.RBTB
    TPAD = NCALL * GCH

    sup_order = pa["sup_order"]
    em = pa["em"]
    tile_of_em = pa["tile_of_em"]
    grp_order = pa["grp_order"]
    grank = pa["rank"]
    per_lvl = C.NG // C.NLV

    # PSUM region per dst tile: super-group sg, local tl -> bank 2+tl//4,
    # col (tl%4)*128
    def _region(t):
        for sgi, tiles in enumerate(C.SGS):
            if t in tiles:
                tl = t - tiles[0]
                return sgi, 2 + tl // 4, (tl % 4) * 128
        raise AssertionError(t)

    # ACT relu order: per sg, banks ascending, 4 tiles each (em order of
    # regions). rlu counter increments once per tile after relu (+rezero).
    act_order = []             # (sg, bank, col, t, rezero)
    for sgi, tiles in enumerate(C.SGS):
        by_bank = {}
        for t in tiles:
            _, b, col = _region(t)
            by_bank.setdefault(b, []).append((col, t))
        for b in sorted(by_bank):
            for col, t in sorted(by_bank[b]):
                act_order.append((sgi, b, col, t, sgi + 1 < len(C.SGS)))
    # rlu threshold for tensor: before FIRST touch of bank b in sg>=1, wait
    # until all of sg-1's tiles on bank b are relu'd+rezeroed. rlu counts
    # REZERO completions only (in act_order order; non-rz tiles don't inc).
    rlu_thresh = {}
    nrz = 0
    for (sgi, b, col, t, rz) in act_order:
        if rz:
            nrz += 1
            rlu_thresh[(sgi + 1, b)] = nrz
    # per-(sg,bank) peB threshold for ACT: max k_last over the bank's tiles
    peb_thresh = {}
    for (sgi, b, col, t, rz) in act_order:
        key = (sgi, b)
        peb_thresh[key] = max(peb_thresh.get(key, 0), int(k_last[t]) + 1)

    nc = bacc.Bacc("TRN2", num_swdge_queues=C.NQ)
    hT = nc.declare_dram_parameter("hT", [128, C.NP2], bf16, isOutput=False)
    xT = nc.declare_dram_parameter("xT", [128, C.NP2], bf16, isOutput=False)
    v0d = nc.declare_dram_parameter("v0", [128, SW], bf16, isOutput=False)
    v1d = nc.declare_dram_parameter("v1", [128, SW], bf16, isOutput=False)
    eidxd = nc.declare_dram_parameter("eidx", [128, T16], i16, isOutput=False)
    dlocd = nc.declare_dram_parameter("dloc", [128, TPAD], f32, isOutput=False)
    valsd = nc.declare_dram_parameter("vals", [128, TPAD], f32, isOutput=False)
    outd = nc.declare_dram_parameter("out", [C.NPAD, 128], f32, isOutput=True)
    pre2 = nc.dram_tensor("pre2", [C.NP2, SW], bf16)

    with ExitStack() as ctx:
        ec = ctx.enter_context
        # SBUF
        hx_sb = [[ec(nc.sbuf_tensor(f"hx{b}{k}", [128, C.SUP], bf16))
                  for k in range(2)] for b in range(2)]
        v_sb = [ec(nc.sbuf_tensor(f"v{k}_sb", [128, SW], bf16)) for k in range(2)]
        eidx_sb = ec(nc.sbuf_tensor("eidx_sb", [128, T16], i16))
        pout_sb = [[ec(nc.sbuf_tensor(f"pout{h}{b}", [128, C.GRP, SW], bf16))
                    for b in range(2)] for h in range(2)]
        msg_sb = [ec(nc.sbuf_tensor(f"msg{b}", [128, GCH, 128], bf16))
                  for b in range(RB)]
        tb_sb = ec(nc.sbuf_tensor("tb_sb", [128, RBTB * GCH, 128], bf16))
        dloc_sb = ec(nc.sbuf_tensor("dloc_sb", [128, TPAD], f32))
        vals_sb = ec(nc.sbuf_tensor("vals_sb", [128, TPAD], f32))
        zero_sb = ec(nc.sbuf_tensor("zero_sb", [128, 128], f32))
        outb_sb = [ec(nc.sbuf_tensor(f"ob{b}", [128, 128], f32)) for b in range(4)]
        # PSUM: 8 full banks; 0/1 phase A, 2..7 hold 4 dst regions each
        psum = [ec(nc.psum_tensor(f"ps{b}", [128, 512], f32)) for b in range(8)]
        # semaphores
        in_sem = ec(nc.semaphore("in_sem"))
        hxs = [ec(nc.semaphore(f"hxs{i}")) for i in range(2)]
        p2wd = ec(nc.semaphore("p2wd"))
        gths = [ec(nc.semaphore(f"gths{i}")) for i in range(RB)]
        dve_c = ec(nc.semaphore("dve_c"))
        outws = [ec(nc.semaphore(f"outws{i}")) for i in range(4)]
        mz = ec(nc.semaphore("mz"))
        zps = ec(nc.semaphore("zps"))
        rel = ec(nc.semaphore("rel"))
        mmA = ec(nc.semaphore("mmA"))
        cpy = ec(nc.semaphore("cpy"))
        peB = ec(nc.semaphore("peB"))
        rlu = ec(nc.semaphore("rlu"))

        with nc.Block() as block:

            @block.sync
            def _(sync):
                sync.dma_start(out=v_sb[0][:], in_=v0d[:]).then_inc(in_sem, 16)
                sync.dma_start(out=v_sb[1][:], in_=v1d[:]).then_inc(in_sem, 16)
                sync.dma_start(out=eidx_sb[:], in_=eidxd[:]).then_inc(in_sem, 16)
                sync.dma_start(out=dloc_sb[:], in_=dlocd[:]).then_inc(in_sem, 16)
                sync.dma_start(out=vals_sb[:], in_=valsd[:]).then_inc(in_sem, 16)
                for pos in range(C.NSUP):
                    if pos >= 2:
                        sync.wait_ge(mmA, C.T20 * (pos - 1))
                    sup = sup_order[pos]
                    sl = slice(sup * C.SUP, (sup + 1) * C.SUP)
                    sync.dma_start(out=hx_sb[pos % 2][0][:],
                                   in_=hT[:, sl]).then_inc(hxs[pos % 2], 16)
                    sync.dma_start(out=hx_sb[pos % 2][1][:],
                                   in_=xT[:, sl]).then_inc(hxs[pos % 2], 16)

            # interleave schedule: which early calls run between phase-A
            # supers (level-gated so their pre2 data exists)
            # level lv ready once group rank per_lvl*(lv+1)-1 is DMA'd; use
            # its em_last + margin as the earliest super position.
            em_need = [pa["em_last"][grp_order[per_lvl * (lv + 1) - 1]]
                       for lv in range(C.NLV)]
            ILV_CAP = 120
            ILV_TAKE = 6
            ilv_sched = {}
            ci = 0
            for pos in range(C.NSUP):
                avail = -1
                for lv in range(C.NLV):
                    if em_need[lv] + 12 <= (pos + 1) * C.T20:
                        avail = lv
                take = []
                while (ci < min(ILV_CAP, NCALL) and len(take) < ILV_TAKE
                       and calls[ci]["q"] <= avail):
                    take.append(ci)
                    ci += 1
                if take:
                    ilv_sched[pos] = take
            n_ilv = ci

            @block.tensor
            def _(tensor):
                state = dict(init=False, bank_seen=set())

                def _emit_call(ci):
                    cl = calls[ci]
                    if not state["init"]:
                        tensor.wait_ge(mz, RB)
                        tensor.wait_ge(zps, 7)
                        state["init"] = True
                    tensor.wait_ge(gths[ci % RB], 16 * (ci // RB + 1))
                    tensor.wait_ge(dve_c, ci + 1)
                    for k in range(cl["n_sub"]):
                        i = cl["sub0"] + k
                        t = int(sub_tile[i])
                        sgi, bank, col = _region(t)
                        if sgi >= 1 and (sgi, bank) not in state["bank_seen"]:
                            state["bank_seen"].add((sgi, bank))
                            tensor.wait_ge(rlu, rlu_thresh[(sgi, bank)])
                        mm = tensor.matmul(
                            psum[bank][:, col:col + 128],
                            tb_sb[:, (ci % RBTB) * GCH + k, :],
                            msg_sb[ci % RB][:, k, :],
                            start=False, stop=(i in last_sub),
                            skip_group_check=True,
                        )
                        if k == cl["n_sub"] - 1:
                            mm.then_inc(peB)

                # ---- phase A (banks 0/1) with interleaved early calls ----
                tensor.wait_ge(in_sem, 80)
                for pos in range(C.NSUP):
                    tensor.wait_ge(hxs[pos % 2], 32 * (pos // 2 + 1))
                    for t20 in range(C.T20):
                        e = pos * C.T20 + t20
                        if e >= 2:
                            tensor.wait_ge(cpy, e - 1)
                        sl = slice(t20 * 128, (t20 + 1) * 128)
                        tensor.matmul(psum[e % 2][:, :], hx_sb[pos % 2][0][:, sl],
                                      v_sb[0][:], start=True, stop=False,
                                      skip_group_check=True)
                        tensor.matmul(psum[e % 2][:, :], hx_sb[pos % 2][1][:, sl],
                                      v_sb[1][:], start=False, stop=True,
                                      skip_group_check=True).then_inc(mmA)
                    for cix in ilv_sched.get(pos, []):
                        _emit_call(cix)
                # ---- phase B ----
                for cix in range(n_ilv, NCALL):
                    _emit_call(cix)

            @block.scalar
            def _(scalar):
                Copy = mybir.ActivationFunctionType.Copy
                Relu = mybir.ActivationFunctionType.Relu
                # ---- phase A: psum -> bf16 pout (strict emission order) ----
                # one DMA per completed 8-tile group (within-half-contiguous);
                # group DMAs complete in issue order (same HWDGE ring).
                for e in range(C.NTA):
                    j = int(tile_of_em[e])
                    h, jh = j // C.NTH, j % C.NTH
                    gi, q8 = jh // C.GRP, jh % C.GRP
                    if q8 == 0 and gi >= 2:
                        scalar.wait_ge(p2wd, 16 * (grank[(h, gi - 2)] + 1))
                    scalar.wait_ge(mmA, e + 1)
                    scalar.activation(pout_sb[h][gi % 2][:, q8, :],
                                      psum[e % 2][:, :], Copy).then_inc(cpy)
                    if q8 == C.GRP - 1:
                        scalar.wait_ge(cpy, e + 1)
                        ph = pre2[h * C.HALF:(h + 1) * C.HALF, :].rearrange(
                            "(p j) c -> p j c", p=128)
                        scalar.dma_start(
                            out=ph[:, gi * C.GRP:(gi + 1) * C.GRP, :],
                            in_=pout_sb[h][gi % 2][:, :, :]).then_inc(p2wd, 16)
                # ---- phase B: relu psum regions -> out, rezero regions ----
                scalar.wait_ge(zps, 7)     # zero_sb ready
                r = 0
                nouts = [0, 0, 0, 0]
                done_bank = set()
                for (sgi, bank, col, t, rz) in act_order:
                    if (sgi, bank) not in done_bank:
                        done_bank.add((sgi, bank))
                        scalar.wait_ge(peB, peb_thresh[(sgi, bank)])
                    if r >= 4:
                        scalar.wait_ge(outws[r % 4], 16 * (r // 4))
                    scalar.activation(outb_sb[r % 4][:],
                                      psum[bank][:, col:col + 128],
                                      Relu).then_inc(rel)
                    scalar.wait_ge(rel, r + 1)
                    if rz:
                        scalar.activation(psum[bank][:, col:col + 128],
                                          zero_sb[:], Copy).then_inc(rlu)
                    scalar.dma_start(out=outd[t * 128:(t + 1) * 128, :],
                                     in_=outb_sb[r % 4][:]).then_inc(outws[r % 4], 16)
                    nouts[r % 4] += 1
                    r += 1
                for q in range(4):
                    if nouts[q]:
                        scalar.wait_ge(outws[q], 16 * nouts[q])

            @block.vector
            def _(vector):
                onehot = _register_onehot_op()
                # zero-init phase-B psum banks (values; matmuls use
                # accumulate-or-overwrite so either bit state is correct)
                vector.memset(zero_sb[:], 0.0).then_inc(zps)
                for b in range(2, 8):
                    vector.memset(psum[b][:, :], 0.0).then_inc(zps)
                vector.wait_ge(in_sem, 80)
                for b in range(NCALL):
                    ns = calls[b]["n_sub"]
                    if b >= RBTB:
                        vector.wait_ge(peB, b - RBTB + 1)
                    s0 = (b % RBTB) * GCH
                    in0 = vals_sb[:, b * GCH:b * GCH + ns].unsqueeze(
                        2).broadcast_to((128, ns, 128))
                    in1 = dloc_sb[:, b * GCH:b * GCH + ns].unsqueeze(
                        2).broadcast_to((128, ns, 128))
                    vector._custom_dve(
                        onehot, out=tb_sb[:, s0:s0 + ns, :],
                        in0=in0, in1=in1).then_inc(dve_c)

            @block.gpsimd
            def _(gpsimd):
                gpsimd.load_library(mlp)
                gpsimd.wait_ge(in_sem, 80)
                for b in range(RB):
                    gpsimd.memzero(msg_sb[b][:]).then_inc(mz)
                gpsimd.wait_ge(mz, RB)
                nreg = nc.alloc_register(mybir.EngineType.Pool, "nidx")
                cur_lvl = -1
                for ci, cl in enumerate(calls):
                    s, h = cl["s"], cl["h"]
                    if cl["q"] > cur_lvl:
                        cur_lvl = cl["q"]
                        gpsimd.wait_ge(p2wd, 16 * per_lvl * (cur_lvl + 1))
                    if ci >= RB:
                        gpsimd.wait_ge(peB, ci - RB + 1)
                    n_idx = cl["n_sub"] * 128
                    src_ap = pre2[h * C.HALF:(h + 1) * C.HALF,
                                  s * 128:(s + 1) * 128]
                    o16 = cl["sub0"] * 8  # *128/16
                    gpsimd.reg_mov(nreg, cl["nireg"])
                    gpsimd.dma_gather(
                        msg_sb[ci % RB][:, 0:cl["n_sub"], :],
                        src_ap,
                        eidx_sb[:, o16:o16 + n_idx // 16],
                        n_idx, nreg, 128, elem_step=SW,
                        queue_num=ci % C.NQ,
                    ).then_inc(gths[ci % RB], 16)

    nc.compile()
    return nc


# ------------------------------------------------------------ entrypoint ---

_CACHE = {}


def _get_graph(C, sched_key, pa, sched):
    if sched_key not in _CACHE:
        _CACHE[sched_key] = _build_graph(C, pa, sched)
    return _CACHE[sched_key]


def _host_prep(h, x, W, inp_W, mix_w, inp_mix_w, edge_val, edge_src, edge_dst, C):
    v0, v1 = _fold_weights(np.asarray(W), np.asarray(inp_W),
                           np.asarray(mix_w), np.asarray(inp_mix_w), C)
    hTf = np.zeros((128, C.NP2), dtype=BF16)
    xTf = np.zeros((128, C.NP2), dtype=BF16)
    hTf[:, :C.N] = np.asarray(h, dtype=np.float32).T.astype(BF16)
    xTf[:, :C.N] = np.asarray(x, dtype=np.float32).T.astype(BF16)
    pa = _phase_a_meta(C)
    sched, per_core = _prep_edges(np.asarray(edge_src), np.asarray(edge_dst),
                                  np.asarray(edge_val, dtype=np.float32), C, pa)
    in_maps = []
    for c in range(C.M):
        in_maps.append(dict(
            hT=np.ascontiguousarray(hTf), xT=np.ascontiguousarray(xTf),
            v0=v0, v1=v1,
            eidx=np.ascontiguousarray(per_core[c]["eidx"]),
            dloc=per_core[c]["dloc"], vals=per_core[c]["vals"],
        ))
    return pa, sched, in_maps


def kernel(h, x, W, inp_W, mix_w, inp_mix_w, edge_val, edge_src, edge_dst,
           _cfg=None, _trace=False):
    C = _cfg or FULL
    pa, sched, in_maps = _host_prep(h, x, W, inp_W, mix_w, inp_mix_w,
                                    edge_val, edge_src, edge_dst, C)
    key = (C.N, C.E, sched["TOT"], sched["NSUB"])
    nc = _get_graph(C, key, pa, sched)

    from concourse.bass_utils import run_bass_kernel_spmd
    res = run_bass_kernel_spmd(nc, in_maps, core_ids=list(range(C.M)),
                               trace=_trace)
    out = np.empty((C.N, 128), dtype=np.float32)
    for c in range(C.M):
        out[c * C.NSH:(c + 1) * C.NSH] = res.results[c]["out"][:C.NSH]
    kernel._last_exec_ns = res.exec_time_ns
    return out


# revision 14
# speedup vs baseline: 1.8671x; 1.0217x over previous
"""Distributed Trainium2 kernel for AdaptiveGraphRecursiveConvolution.

Math (reference):
    out = relu( sum_g mix_w[g] * sum_k A_{gk} @ (h @ W[g,k])
              + sum_g inp_mix_w[g] * sum_k A_{gk} @ (x @ inp_W[g,k]) )

Folding the scalar mixing weights into the dense weights and merging the
h/x paths gives, with S = G*K edge sets and V_s = [mix_w*W_s ; inp_mix*inp_W_s]:
    out = relu( sum_s A_s @ (hx @ V_s) ),  hx = [h | x]  (N x 2F)

Device strategy (8 NeuronCores, SPMD single graph, per-core data):
  - dst-nodes sharded: core c owns rows [c*NSH, (c+1)*NSH).
  - Phase A (replicated): pre2 = hx @ [V_0..V_3]  ([NP2, S*128] bf16) via PE,
    streamed to private DRAM. Node supertiles are emitted with the two
    int16-index halves INTERLEAVED so both halves become gatherable early.
  - Phase B: per edge-set SpMM. Edges (pre-sharded/sorted/padded on host) are
    gathered per-edge from pre2 via SWDGE dma_gather (bf16 rows), and
    scatter-added into PSUM dst regions via one-hot matmuls on PE:
        psum[region(dst_tile)] += T_chunk^T @ msg_chunk
    where T[e, j] = val_e * (dst_loc_e == j) is built on DVE with one fused
    tensor_scalar (is_equal then mult) per 128-edge subchunk.
  - PSUM packing: 4 dst tiles per bank (regions), matmuls never use
    start=True on phase-B banks; regions are value-zeroed before reuse so
    accumulate-or-overwrite is correct either way. Tiles are grouped in
    super-groups of 24 (6 banks x 4 regions); calls within a super-group are
    sorted by pre2-readiness level so phase B streams during phase A.
  - relu on ACT from PSUM, DMA out. Host concatenates the 8 shards.
"""

import os
import sys
import time

import numpy as np

sys.path.insert(0, "/opt/trn_rl_repo")
sys.path.insert(0, "/root/.axon_site/_ro/trn_rl_repo")

import ml_dtypes  # noqa: E402

BF16 = ml_dtypes.bfloat16

_ONEHOT = None


def _register_onehot_op():
    """Custom DVE op: out[p, t] = (t == in1[p, t]) * in0[p, t], with t the
    element position along the free dims. With in1 = dlocg (128*k + dstloc,
    per-subchunk column broadcast along the last dim) and in0 = vals
    broadcast, one instruction builds the scaled one-hot T block for a whole
    gather call."""
    global _ONEHOT
    if _ONEHOT is not None:
        return _ONEHOT
    from concourse import dve_ops
    from concourse.dve_spec import Spec, Src0, Src1, Idx, eq, lower
    from concourse.dve_uop import DveOpSpec

    NAME = "ONEHOT_VAL_ANT"
    if NAME in dve_ops._SUB_OPCODE_FOR_NAME:
        _ONEHOT = next(o for o in dve_ops.OPS if o.name == NAME)
        return _ONEHOT

    def _ref(in0, in1, s0, s1, imm2):
        P = in1.shape[0]
        f1 = np.asarray(in1, np.float32).reshape(P, -1)
        f0 = np.asarray(in0, np.float32).reshape(P, -1)
        idx = np.arange(f1.shape[1], dtype=np.float32)[None, :]
        return (idx == f1).astype(np.float32) * f0

    spec = Spec(body=eq(Idx, Src1) * Src0, reference=_ref)
    opc = max(dve_ops._SUB_OPCODE_FOR_NAME.values()) + 1
    assert opc < 0x20
    shas = {}
    for ver in ("v3", "v4"):
        s = DveOpSpec(name=NAME, opcode=opc, uops=lower(spec, ver=ver),
                      rd1_en=True)
        shas[ver] = s.sha(ver)
    op = dve_ops.DveOp(NAME, spec, subdim=False, uops_sha=shas)
    dve_ops.OPS.append(op)
    dve_ops.CUSTOM_DVE_SPECS[NAME] = spec
    dve_ops._SUB_OPCODE_FOR_NAME[NAME] = opc
    _ONEHOT = op
    return op


# ---------------------------------------------------------------- config ---

class Cfg:
    def __init__(self, N, E, S=4, M=8, HALF=None, SUP=None, GCH=8, SGT=24,
                 RB=12, RBTB=12, NQ=4, NLV=8):
        self.N, self.E, self.S, self.M = N, E, S, M
        self.F2 = 256          # hx feature dim (2*128)
        self.O = 128
        self.NSH = N // M      # dst rows owned per core
        self.NPAD = _ru(self.NSH, 128)
        self.NT = self.NPAD // 128          # dst tiles per core
        self.SGT = SGT                      # tiles per super-group (<= 24)
        self.SGS = [list(range(a, min(a + SGT, self.NT)))
                    for a in range(0, self.NT, SGT)]
        self.HALF = HALF if HALF else _ru(-(-N // 2), 128)
        assert self.HALF <= 32767
        self.NP2 = 2 * self.HALF            # padded node rows in pre2
        assert N <= self.NP2
        self.SUP = SUP if SUP else 2560     # nodes per phase-A supertile
        assert self.SUP % 128 == 0 and self.NP2 % self.SUP == 0
        self.NSUP = self.NP2 // self.SUP
        assert self.NSUP % 2 == 0
        self.T20 = self.SUP // 128
        self.NTA = self.NP2 // 128          # phase-A node tiles
        self.GCH = GCH                      # max subchunks (128 edges) per gather
        self.RB = RB                        # msg ring depth (calls)
        self.RBTB = RBTB                    # T ring depth (calls)
        self.NQ = NQ                        # SWDGE queues
        self.NLV = NLV                      # pre2 readiness levels
        self.GRP = 8                        # tiles per pre2 write group
        self.NTH = self.HALF // 128         # node tiles per half
        self.NGH = self.NTH // self.GRP     # write groups per half
        self.NG = 2 * self.NGH
        assert self.NG % NLV == 0


def _ru(x, m):
    return (x + m - 1) // m * m


FULL = Cfg(N=40000, E=640000, HALF=20480, SUP=2560)


# ------------------------------------------------------------- host prep ---

def _fold_weights(W, inp_W, mix_w, inp_mix_w, C):
    """Return v0, v1: [128, S*128] bf16 (h-path and x-path stationary weights)."""
    G, K = W.shape[0], W.shape[1]
    S = G * K
    Wm = (W.astype(np.float64) * mix_w.astype(np.float64)[:, None, None, None])
    Im = (inp_W.astype(np.float64) * inp_mix_w.astype(np.float64)[:, None, None, None])
    v0 = Wm.reshape(S, 128, 128).transpose(1, 0, 2).reshape(128, S * 128)
    v1 = Im.reshape(S, 128, 128).transpose(1, 0, 2).reshape(128, S * 128)
    return v0.astype(BF16), v1.astype(BF16)


def _phase_a_meta(C):
    """Emission order of phase-A supertiles/groups with interleaved halves."""
    H = C.NSUP // 2
    sup_order = []
    for k in range(H):
        sup_order += [k, H + k]
    # em[j]: emission index of phase-A node tile j (j = NP2-row-block index)
    em = np.zeros(C.NTA, dtype=np.int64)
    for pos, sup in enumerate(sup_order):
        for t20 in range(C.T20):
            em[sup * C.T20 + t20] = pos * C.T20 + t20
    tile_of_em = np.zeros(C.NTA, dtype=np.int64)
    tile_of_em[em] = np.arange(C.NTA)
    # groups (h, gi): tiles jh = gi*GRP .. +GRP-1 of half h
    groups = [(h, gi) for h in range(2) for gi in range(C.NGH)]
    em_last = {}
    for (h, gi) in groups:
        tiles = [h * C.NTH + gi * C.GRP + q for q in range(C.GRP)]
        em_last[(h, gi)] = int(max(em[t] for t in tiles))
    grp_order = sorted(groups, key=lambda g: em_last[g])
    rank = {g: i for i, g in enumerate(grp_order)}
    per_lvl = C.NG // C.NLV
    lvl = {g: rank[g] // per_lvl for g in groups}
    return dict(sup_order=sup_order, em=em, tile_of_em=tile_of_em,
                grp_order=grp_order, rank=rank, lvl=lvl, em_last=em_last)


def _prep_edges(edge_src, edge_dst, edge_val, C, pa):
    """Shard/sort/pad edges. Returns (sched, per_core)."""
    S, E, M = C.S, C.E, C.M
    src = edge_src.reshape(S, E).astype(np.int64)
    dst = edge_dst.reshape(S, E).astype(np.int64)
    val = edge_val.reshape(S, E).astype(np.float32)

    NTH = C.NTH
    lvl = pa["lvl"]
    # raw[c][s][t][h] = (idx16, dstloc, val); pre2 rows are PERMUTED within
    # each half: node (p=sl%128, j=sl//128) is stored at row p*NTH + j.
    raw = [[[[None, None] for _ in range(C.NT)] for _ in range(S)] for _ in range(M)]
    cnt = np.zeros((M, S, C.NT, 2), dtype=np.int64)
    for s in range(S):
        core_of = dst[s] // C.NSH
        for c in range(M):
            sel = np.nonzero(core_of == c)[0]
            d = dst[s][sel] - c * C.NSH
            t = d // 128
            h = (src[s][sel] >= C.HALF).astype(np.int64)
            # secondary sort by src so low-src subchunks can gather early
            key = (t * 2 + h) * (2 * C.HALF) + src[s][sel]
            order = np.argsort(key, kind="stable")
            sel, d, t, h = sel[order], d[order], t[order], h[order]
            key = key[order]
            bounds = np.searchsorted(key, np.arange(C.NT * 2 + 1) * (2 * C.HALF))
            for ti in range(C.NT):
                for hi in range(2):
                    a, b = bounds[ti * 2 + hi], bounds[ti * 2 + hi + 1]
                    ss = sel[a:b]
                    sl = src[s][ss] - hi * C.HALF
                    raw[c][s][ti][hi] = (
                        ((sl % 128) * NTH + sl // 128).astype(np.int16),
                        (d[a:b] - ti * 128).astype(np.int16),
                        val[s][ss],
                    )
                    cnt[c, s, ti, hi] = b - a

    # common padded lengths
    L = np.maximum(cnt.max(axis=0), 1)
    L = ((L + 127) // 128 * 128)  # [S, NT, 2]

    # Per-(s,t,h) per-subchunk readiness class: max over cores of the pre2
    # write-group LEVEL of the rows the subchunk gathers.
    qcls = {}
    for s in range(S):
        for t in range(C.NT):
            for h in range(2):
                nsub = int(L[s][t][h]) // 128
                cls = []
                for j in range(nsub):
                    m = 0
                    for c in range(M):
                        seg_i = raw[c][s][t][h][0][j * 128:(j + 1) * 128]
                        if len(seg_i):
                            gi_max = int((seg_i % NTH).max()) // C.GRP
                            # rows in this subchunk span groups up to gi_max;
                            # level needed = max level among groups 0..gi_max
                            # (sorted-by-src makes gi_max the binding one, but
                            # levels are not monotone in gi -> take max)
                            need = max(lvl[(h, g)] for g in range(gi_max + 1))
                            m = max(m, need)
                    cls.append(m)
                qcls[(s, t, h)] = cls

    # ---- build calls: per super-group, per (h, s) segment, class-sorted ----
    sub_src = []            # (s, t, h, j) per subchunk, final order
    sub_tile = []
    calls = []              # dict: s, h, sub0, n_sub, q, sg
    for sgi, tiles in enumerate(C.SGS):
        sg_calls = []
        for h in range(2):
            for s in range(S):
                ordered = []
                for t in tiles:
                    for j in range(int(L[s][t][h]) // 128):
                        ordered.append((qcls[(s, t, h)][j], t, j))
                ordered.sort()
                o = 0
                while o < len(ordered):
                    take = min(C.GCH, len(ordered) - o)
                    chunk = ordered[o:o + take]
                    sg_calls.append(dict(
                        s=s, h=h, sg=sgi, subs=chunk,
                        q=max(cl for cl, _, _ in chunk)))
                    o += take
        sg_calls.sort(key=lambda cl: cl["q"])
        for cl in sg_calls:
            cl["sub0"] = len(sub_src)
            cl["n_sub"] = len(cl["subs"])
            for (_, t, j) in cl["subs"]:
                sub_src.append((cl["s"], t, cl["h"], j))
                sub_tile.append(t)
            calls.append(cl)
    NSUB = len(sub_src)
    TOT = NSUB * 128
    assert TOT == int(L.sum())

    # Per-subchunk max real count over cores (for trailing-pad skip)
    def _nreal(sc):
        s, t, h, j = sc
        m = 0
        for c in range(M):
            m = max(m, min(128, max(0, len(raw[c][s][t][h][0]) - j * 128)))
        return m

    # within each call, move the subchunk with most skippable trailing pads
    # to the end; record the call's real (non-skipped) index count
    for cl in calls:
        a, b = cl["sub0"], cl["sub0"] + cl["n_sub"]
        pads = [128 - _nreal(sub_src[i]) for i in range(a, b)]
        kbest = int(np.argmax(pads))
        sub_src[a + kbest:b] = sub_src[a + kbest + 1:b] + [sub_src[a + kbest]]
        st = list(sub_tile[a:b])
        st[kbest:] = st[kbest + 1:] + [st[kbest]]
        sub_tile[a:b] = st
        cl["nireg"] = cl["n_sub"] * 128 - pads[kbest]

    # last subchunk per tile and the call index containing it
    last_sub = np.full(C.NT, -1, dtype=np.int64)
    for i, t in enumerate(sub_tile):
        last_sub[t] = i
    assert (last_sub >= 0).all()
    sub_call = np.zeros(NSUB, dtype=np.int64)
    for ci, cl in enumerate(calls):
        sub_call[cl["sub0"]: cl["sub0"] + cl["n_sub"]] = ci
    k_last = sub_call[last_sub]    # call index of each tile's last subchunk

    sched = dict(L=L, calls=calls, sub_tile=sub_tile, TOT=TOT,
                 NSUB=NSUB, last_sub=last_sub, k_last=k_last,
                 sub_call=sub_call)

    # per-core flattened arrays (slot layout follows sub_src permutation)
    per_core = []
    for c in range(M):
        idx = np.zeros(TOT, dtype=np.int16)
        dl = np.zeros(TOT, dtype=np.int64)
        vl = np.zeros(TOT, dtype=np.float32)
        for i, (s, t, h, j) in enumerate(sub_src):
            i16, d16, v32 = raw[c][s][t][h]
            a, b = j * 128, min((j + 1) * 128, len(i16))
            n = max(0, b - a)
            o = i * 128
            if n > 0:
                idx[o:o + n] = i16[a:b]
                dl[o:o + n] = d16[a:b]
                vl[o:o + n] = v32[a:b]
        for cl in calls:
            oe = (cl["sub0"] + cl["n_sub"]) * 128
            skip = cl["n_sub"] * 128 - cl["nireg"]
            if skip:
                idx[oe - skip:oe] = -1
        eidx = np.tile(idx.reshape(TOT // 16, 16).T, (8, 1))      # [128, TOT/16]
        # per-subchunk (dstloc, val) sidebands, CALL-ALIGNED columns
        NCALL = len(calls)
        dloc = np.zeros((128, NCALL * C.GCH), dtype=np.float32)
        vals = np.zeros((128, NCALL * C.GCH), dtype=np.float32)
        dl2 = dl.reshape(TOT // 128, 128).T.astype(np.float32)
        vl2 = vl.reshape(TOT // 128, 128).T
        koff = np.arange(C.GCH, dtype=np.float32) * 128.0
        for b, cl in enumerate(calls):
            a0, ns = cl["sub0"], cl["n_sub"]
            dloc[:, b * C.GCH:b * C.GCH + ns] = (dl2[:, a0:a0 + ns]
                                                 + koff[None, :ns])
            vals[:, b * C.GCH:b * C.GCH + ns] = vl2[:, a0:a0 + ns]
        per_core.append(dict(eidx=eidx, dloc=dloc, vals=vals))
    return sched, per_core


# ----------------------------------------------------------- graph build ---

def _build_graph(C, pa, sched):
    import concourse.bass as bass
    import concourse.bacc as bacc
    import concourse.mybir as mybir
    from concourse.library_config import mlp
    from contextlib import ExitStack

    f32, bf16, i16 = mybir.dt.float32, mybir.dt.bfloat16, mybir.dt.int16
    S = C.S
    SW = S * 128                       # pre2 row width
    TOT = sched["TOT"]
    T16 = TOT // 16
    calls = sched["calls"]
    sub_tile = sched["sub_tile"]
    last_sub = set(sched["last_sub"].tolist())
    k_last = sched["k_last"]
    NCALL = len(calls)
    GCH, RB, RBTB = C.GCH, C.RB, C.RBTB
    TPAD = NCALL * GCH

    sup_order = pa["sup_order"]
    em = pa["em"]
    tile_of_em = pa["tile_of_em"]
    grp_order = pa["grp_order"]
    grank = pa["rank"]
    per_lvl = C.NG // C.NLV

    # PSUM region per dst tile: super-group sg, local tl -> bank 2+tl//4,
    # col (tl%4)*128
    def _region(t):
        for sgi, tiles in enumerate(C.SGS):
            if t in tiles:
                tl = t - tiles[0]
                return sgi, 2 + tl // 4, (tl % 4) * 128
        raise AssertionError(t)

    # ACT relu order: per sg, banks ascending, 4 tiles each (em order of
    # regions). rlu counter increments once per tile after relu (+rezero).
    act_order = []             # (sg, bank, col, t, rezero)
    for sgi, tiles in enumerate(C.SGS):
        by_bank = {}
        for t in tiles:
            _, b, col = _region(t)
            by_bank.setdefault(b, []).append((col, t))
        for b in sorted(by_bank):
            for col, t in sorted(by_bank[b]):
                act_order.append((sgi, b, col, t, sgi + 1 < len(C.SGS)))
    # rlu threshold for tensor: before FIRST touch of bank b in sg>=1, wait
    # until all of sg-1's tiles on bank b are relu'd+rezeroed. rlu counts
    # REZERO completions only (in act_order order; non-rz tiles don't inc).
    rlu_thresh = {}
    nrz = 0
    for (sgi, b, col, t, rz) in act_order:
        if rz:
            nrz += 1
            rlu_thresh[(sgi + 1, b)] = nrz
    # per-(sg,bank) peB threshold for ACT: max k_last over the bank's tiles
    peb_thresh = {}
    for (sgi, b, col, t, rz) in act_order:
        key = (sgi, b)
        peb_thresh[key] = max(peb_thresh.get(key, 0), int(k_last[t]) + 1)

    nc = bacc.Bacc("TRN2", num_swdge_queues=C.NQ)
    hT = nc.declare_dram_parameter("hT", [128, C.NP2], bf16, isOutput=False)
    xT = nc.declare_dram_parameter("xT", [128, C.NP2], bf16, isOutput=False)
    v0d = nc.declare_dram_parameter("v0", [128, SW], bf16, isOutput=False)
    v1d = nc.declare_dram_parameter("v1", [128, SW], bf16, isOutput=False)
    eidxd = nc.declare_dram_parameter("eidx", [128, T16], i16, isOutput=False)
    dlocd = nc.declare_dram_parameter("dloc", [128, TPAD], f32, isOutput=False)
    valsd = nc.declare_dram_parameter("vals", [128, TPAD], f32, isOutput=False)
    outd = nc.declare_dram_parameter("out", [C.NPAD, 128], f32, isOutput=True)
    pre2 = nc.dram_tensor("pre2", [C.NP2, SW], bf16)

    with ExitStack() as ctx:
        ec = ctx.enter_context
        # SBUF
        hx_sb = [[ec(nc.sbuf_tensor(f"hx{b}{k}", [128, C.SUP], bf16))
                  for k in range(2)] for b in range(2)]
        v_sb = [ec(nc.sbuf_tensor(f"v{k}_sb", [128, SW], bf16)) for k in range(2)]
        eidx_sb = ec(nc.sbuf_tensor("eidx_sb", [128, T16], i16))
        pout_sb = [[ec(nc.sbuf_tensor(f"pout{h}{b}", [128, C.GRP, SW], bf16))
                    for b in range(2)] for h in range(2)]
        msg_sb = [ec(nc.sbuf_tensor(f"msg{b}", [128, GCH, 128], bf16))
                  for b in range(RB)]
        tb_sb = ec(nc.sbuf_tensor("tb_sb", [128, RBTB * GCH, 128], bf16))
        dloc_sb = ec(nc.sbuf_tensor("dloc_sb", [128, TPAD], f32))
        vals_sb = ec(nc.sbuf_tensor("vals_sb", [128, TPAD], f32))
        zero_sb = ec(nc.sbuf_tensor("zero_sb", [128, 128], f32))
        outb_sb = [ec(nc.sbuf_tensor(f"ob{b}", [128, 128], f32)) for b in range(4)]
        # PSUM: 8 full banks; 0/1 phase A, 2..7 hold 4 dst regions each
        psum = [ec(nc.psum_tensor(f"ps{b}", [128, 512], f32)) for b in range(8)]
        # semaphores
        in_sem = ec(nc.semaphore("in_sem"))
        hxs = [ec(nc.semaphore(f"hxs{i}")) for i in range(2)]
        p2wd = ec(nc.semaphore("p2wd"))
        gths = [ec(nc.semaphore(f"gths{i}")) for i in range(RB)]
        dve_c = ec(nc.semaphore("dve_c"))
        outws = [ec(nc.semaphore(f"outws{i}")) for i in range(4)]
        mz = ec(nc.semaphore("mz"))
        zps = ec(nc.semaphore("zps"))
        rel = ec(nc.semaphore("rel"))
        eix = ec(nc.semaphore("eix"))
        dvs = ec(nc.semaphore("dvs"))
        mmA = ec(nc.semaphore("mmA"))
        cpy = ec(nc.semaphore("cpy"))
        peB = ec(nc.semaphore("peB"))
        rlu = ec(nc.semaphore("rlu"))

        with nc.Block() as block:

            @block.sync
            def _(sync):
                sync.dma_start(out=eidx_sb[:], in_=eidxd[:]).then_inc(eix, 16)
                sync.dma_start(out=v_sb[0][:], in_=v0d[:]).then_inc(in_sem, 16)
                sync.dma_start(out=v_sb[1][:], in_=v1d[:]).then_inc(in_sem, 16)
                sync.dma_start(out=dloc_sb[:], in_=dlocd[:]).then_inc(dvs, 16)
                sync.dma_start(out=vals_sb[:], in_=valsd[:]).then_inc(dvs, 16)
                for pos in range(C.NSUP):
                    if pos >= 2:
                        sync.wait_ge(mmA, C.T20 * (pos - 1))
                    sup = sup_order[pos]
                    sl = slice(sup * C.SUP, (sup + 1) * C.SUP)
                    sync.dma_start(out=hx_sb[pos % 2][0][:],
                                   in_=hT[:, sl]).then_inc(hxs[pos % 2], 16)
                    sync.dma_start(out=hx_sb[pos % 2][1][:],
                                   in_=xT[:, sl]).then_inc(hxs[pos % 2], 16)

            # interleave schedule: which early calls run between phase-A
            # supers (level-gated so their pre2 data exists)
            # level lv ready once group rank per_lvl*(lv+1)-1 is DMA'd; use
            # its em_last + margin as the earliest super position.
            em_need = [pa["em_last"][grp_order[per_lvl * (lv + 1) - 1]]
                       for lv in range(C.NLV)]
            ILV_CAP = 170
            ILV_TAKE = 7
            ilv_sched = {}
            ci = 0
            for pos in range(C.NSUP):
                avail = -1
                for lv in range(C.NLV):
                    if em_need[lv] + 12 <= (pos + 1) * C.T20:
                        avail = lv
                take = []
                while (ci < min(ILV_CAP, NCALL) and len(take) < ILV_TAKE
                       and calls[ci]["q"] <= avail):
                    take.append(ci)
                    ci += 1
                if take:
                    ilv_sched[pos] = take
            n_ilv = ci

            @block.tensor
            def _(tensor):
                state = dict(init=False, bank_seen=set())

                def _emit_call(ci):
                    cl = calls[ci]
                    if not state["init"]:
                        tensor.wait_ge(mz, RB)
                        tensor.wait_ge(zps, 7)
                        state["init"] = True
                    tensor.wait_ge(gths[ci % RB], 16 * (ci // RB + 1))
                    tensor.wait_ge(dve_c, ci + 1)
                    for k in range(cl["n_sub"]):
                        i = cl["sub0"] + k
                        t = int(sub_tile[i])
                        sgi, bank, col = _region(t)
                        if sgi >= 1 and (sgi, bank) not in state["bank_seen"]:
                            state["bank_seen"].add((sgi, bank))
                            tensor.wait_ge(rlu, rlu_thresh[(sgi, bank)])
                        mm = tensor.matmul(
                            psum[bank][:, col:col + 128],
                            tb_sb[:, (ci % RBTB) * GCH + k, :],
                            msg_sb[ci % RB][:, k, :],
                            start=False, stop=(i in last_sub),
                            skip_group_check=True,
                        )
                        if k == cl["n_sub"] - 1:
                            mm.then_inc(peB)

                # ---- phase A (banks 0/1) with interleaved early calls ----
                tensor.wait_ge(in_sem, 32)
                for pos in range(C.NSUP):
                    tensor.wait_ge(hxs[pos % 2], 32 * (pos // 2 + 1))
                    for t20 in range(C.T20):
                        e = pos * C.T20 + t20
                        if e >= 2:
                            tensor.wait_ge(cpy, e - 1)
                        sl = slice(t20 * 128, (t20 + 1) * 128)
                        tensor.matmul(psum[e % 2][:, :], hx_sb[pos % 2][0][:, sl],
                                      v_sb[0][:], start=True, stop=False,
                                      skip_group_check=True)
                        tensor.matmul(psum[e % 2][:, :], hx_sb[pos % 2][1][:, sl],
                                      v_sb[1][:], start=False, stop=True,
                                      skip_group_check=True).then_inc(mmA)
                    for cix in ilv_sched.get(pos, []):
                        _emit_call(cix)
                # ---- phase B ----
                for cix in range(n_ilv, NCALL):
                    _emit_call(cix)

            @block.scalar
            def _(scalar):
                Copy = mybir.ActivationFunctionType.Copy
                Relu = mybir.ActivationFunctionType.Relu
                # ---- phase A: psum -> bf16 pout (strict emission order) ----
                # one DMA per completed 8-tile group (within-half-contiguous);
                # group DMAs complete in issue order (same HWDGE ring).
                for e in range(C.NTA):
                    j = int(tile_of_em[e])
                    h, jh = j // C.NTH, j % C.NTH
                    gi, q8 = jh // C.GRP, jh % C.GRP
                    if q8 == 0 and gi >= 2:
                        scalar.wait_ge(p2wd, 16 * (grank[(h, gi - 2)] + 1))
                    scalar.wait_ge(mmA, e + 1)
                    scalar.activation(pout_sb[h][gi % 2][:, q8, :],
                                      psum[e % 2][:, :], Copy).then_inc(cpy)
                    if q8 == C.GRP - 1:
                        scalar.wait_ge(cpy, e + 1)
                        ph = pre2[h * C.HALF:(h + 1) * C.HALF, :].rearrange(
                            "(p j) c -> p j c", p=128)
                        scalar.dma_start(
                            out=ph[:, gi * C.GRP:(gi + 1) * C.GRP, :],
                            in_=pout_sb[h][gi % 2][:, :, :]).then_inc(p2wd, 16)
                # ---- phase B: relu psum regions -> out, rezero regions ----
                scalar.wait_ge(zps, 7)     # zero_sb ready
                r = 0
                nouts = [0, 0, 0, 0]
                done_bank = set()
                for (sgi, bank, col, t, rz) in act_order:
                    if (sgi, bank) not in done_bank:
                        done_bank.add((sgi, bank))
                        scalar.wait_ge(peB, peb_thresh[(sgi, bank)])
                    if r >= 4:
                        scalar.wait_ge(outws[r % 4], 16 * (r // 4))
                    scalar.activation(outb_sb[r % 4][:],
                                      psum[bank][:, col:col + 128],
                                      Relu).then_inc(rel)
                    scalar.wait_ge(rel, r + 1)
                    if rz:
                        scalar.activation(psum[bank][:, col:col + 128],
                                          zero_sb[:], Copy).then_inc(rlu)
                    scalar.dma_start(out=outd[t * 128:(t + 1) * 128, :],
                                     in_=outb_sb[r % 4][:]).then_inc(outws[r % 4], 16)
                    nouts[r % 4] += 1
                    r += 1
                for q in range(4):
                    if nouts[q]:
                        scalar.wait_ge(outws[q], 16 * nouts[q])

            @block.vector
            def _(vector):
                onehot = _register_onehot_op()
                # zero-init phase-B psum banks (values; matmuls use
                # accumulate-or-overwrite so either bit state is correct)
                vector.memset(zero_sb[:], 0.0).then_inc(zps)
                for b in range(2, 8):
                    vector.memset(psum[b][:, :], 0.0).then_inc(zps)
                vector.wait_ge(dvs, 32)
                for b in range(NCALL):
                    ns = calls[b]["n_sub"]
                    if b >= RBTB:
                        vector.wait_ge(peB, b - RBTB + 1)
                    s0 = (b % RBTB) * GCH
                    in0 = vals_sb[:, b * GCH:b * GCH + ns].unsqueeze(
                        2).broadcast_to((128, ns, 128))
                    in1 = dloc_sb[:, b * GCH:b * GCH + ns].unsqueeze(
                        2).broadcast_to((128, ns, 128))
                    vector._custom_dve(
                        onehot, out=tb_sb[:, s0:s0 + ns, :],
                        in0=in0, in1=in1).then_inc(dve_c)

            @block.gpsimd
            def _(gpsimd):
                gpsimd.load_library(mlp)
                for b in range(RB):
                    gpsimd.memzero(msg_sb[b][:]).then_inc(mz)
                gpsimd.wait_ge(mz, RB)
                gpsimd.wait_ge(eix, 16)
                nreg = nc.alloc_register(mybir.EngineType.Pool, "nidx")
                cur_lvl = -1
                for ci, cl in enumerate(calls):
                    s, h = cl["s"], cl["h"]
                    if cl["q"] > cur_lvl:
                        cur_lvl = cl["q"]
                        gpsimd.wait_ge(p2wd, 16 * per_lvl * (cur_lvl + 1))
                    if ci >= RB:
                        gpsimd.wait_ge(peB, ci - RB + 1)
                    n_idx = cl["n_sub"] * 128
                    src_ap = pre2[h * C.HALF:(h + 1) * C.HALF,
                                  s * 128:(s + 1) * 128]
                    o16 = cl["sub0"] * 8  # *128/16
                    gpsimd.reg_mov(nreg, cl["nireg"])
                    gpsimd.dma_gather(
                        msg_sb[ci % RB][:, 0:cl["n_sub"], :],
                        src_ap,
                        eidx_sb[:, o16:o16 + n_idx // 16],
                        n_idx, nreg, 128, elem_step=SW,
                        queue_num=ci % C.NQ,
                    ).then_inc(gths[ci % RB], 16)

    nc.compile()
    return nc


# ------------------------------------------------------------ entrypoint ---

_CACHE = {}


def _get_graph(C, sched_key, pa, sched):
    if sched_key not in _CACHE:
        _CACHE[sched_key] = _build_graph(C, pa, sched)
    return _CACHE[sched_key]


def _host_prep(h, x, W, inp_W, mix_w, inp_mix_w, edge_val, edge_src, edge_dst, C):
    v0, v1 = _fold_weights(np.asarray(W), np.asarray(inp_W),
                           np.asarray(mix_w), np.asarray(inp_mix_w), C)
    hTf = np.zeros((128, C.NP2), dtype=BF16)
    xTf = np.zeros((128, C.NP2), dtype=BF16)
    hTf[:, :C.N] = np.asarray(h, dtype=np.float32).T.astype(BF16)
    xTf[:, :C.N] = np.asarray(x, dtype=np.float32).T.astype(BF16)
    pa = _phase_a_meta(C)
    sched, per_core = _prep_edges(np.asarray(edge_src), np.asarray(edge_dst),
                                  np.asarray(edge_val, dtype=np.float32), C, pa)
    in_maps = []
    for c in range(C.M):
        in_maps.append(dict(
            hT=np.ascontiguousarray(hTf), xT=np.ascontiguousarray(xTf),
            v0=v0, v1=v1,
            eidx=np.ascontiguousarray(per_core[c]["eidx"]),
            dloc=per_core[c]["dloc"], vals=per_core[c]["vals"],
        ))
    return pa, sched, in_maps


def kernel(h, x, W, inp_W, mix_w, inp_mix_w, edge_val, edge_src, edge_dst,
           _cfg=None, _trace=False):
    C = _cfg or FULL
    pa, sched, in_maps = _host_prep(h, x, W, inp_W, mix_w, inp_mix_w,
                                    edge_val, edge_src, edge_dst, C)
    key = (C.N, C.E, sched["TOT"], sched["NSUB"])
    nc = _get_graph(C, key, pa, sched)

    from concourse.bass_utils import run_bass_kernel_spmd
    res = run_bass_kernel_spmd(nc, in_maps, core_ids=list(range(C.M)),
                               trace=_trace)
    out = np.empty((C.N, 128), dtype=np.float32)
    for c in range(C.M):
        out[c * C.NSH:(c + 1) * C.NSH] = res.results[c]["out"][:C.NSH]
    kernel._last_exec_ns = res.exec_time_ns
    return out
